# revision 32
# baseline (speedup 1.0000x reference)
"""Distributed attention layer kernel for 8 TRN2 NeuronCores.

Math (per reference): out = softmax_causal((x@Wq)(x@Wk)^T / 8) @ (x@Wv) @ Wo
with B=4, S=2048, D=1024, H=16 heads of dim 64.

Sharding: head tensor-parallel. Core c owns head pair (2c, 2c+1):
  - Wq/Wk/Wv column-sharded [1024, 128]; statesT replicated [1024, 8192].
  - Each core computes qT/kT/vT for its 2 heads, causal attention in
    S^T layout (kj on partitions, qi on free), softmax denominator via a
    ones-column appended to V (PV matmul row 64 = sum of probs).
  - ctx^T tiles are normalized on the fly (reciprocal straight from the
    PSUM denominator row, partition_broadcast on GpSimd, one DVE mul)
    and staged into per-half AllToAll buffers laid out so chunk j =
    [128 head cols, rows of output-core j].
  - Output projection: out_rows = sum_c slab_c.T @ Wo[128c:...] in PSUM.

Scheduling: the PE instruction stream is kept dense to hold the clock
at the top p-state. QKV for batch b+1 is interleaved as filler between
the score/PV matmul pairs of batch b's attention (v-projection first,
so its PSUM->SBUF copy clears the in-order DVE queue long before the
V' transposes need it); the half-0 output projection is interleaved
into batch 3's attention so AllToAll #0 (which also absorbs the
inter-core start skew once) overlaps ~200us of compute. One AllToAll
per half - split collectives serialize on the cc stream and cascade
under start skew. Causal masking of diagonal blocks is a post-exp
multiply by a 0/1 triangular mask (bf16, SBUF) so the score->exp chain
never waits on a PSUM-side DVE add.

Matmul operands are bf16 (PE full rate); accumulation is fp32 in PSUM.
"""

import ml_dtypes
import numpy as np

import concourse.bass as bass
import concourse.mybir as mybir
import concourse.tile as tile
from concourse import bacc
from concourse.masks import make_identity

F32 = mybir.dt.float32
BF16 = mybir.dt.bfloat16

B, S, D, H = 4, 2048, 1024, 16
HD = 64
N_CORES = 8
QI, KJ = 512, 128


def build_tri(KJ=KJ):
    """tri[p, f] = 1.0 if p <= f else 0.0 (valid causal positions of the
    first KJ columns of a diagonal strip)."""
    p = np.arange(KJ)[:, None]
    f = np.arange(KJ)[None, :]
    return np.where(p <= f, 1.0, 0.0).astype(ml_dtypes.bfloat16)


def build(b_=B, s_=S, d_=D, n_cores=N_CORES):
    HPC = d_ // n_cores          # head cols per core (2 heads x 64)
    NH = HPC // HD               # heads per core (2)
    R = b_ * s_                  # global rows (8192)
    Rc = R // n_cores            # output rows per core (1024)
    DT = d_ // 128               # contraction tiles (8)
    SKJ = s_ // KJ               # kj blocks per (b, h) (16)
    SQI = s_ // QI               # qi tiles per (b, h) (4)
    n_half = 2
    BH = b_ // n_half            # batches per half (2)
    HR = Rc // n_half            # rows per core per half (512)
    RH = R // n_half             # global rows per half (4096)
    CL = s_ // QI                # column tiles per batch (4)
    assert s_ % QI == 0 and HR == QI and d_ % 128 == 0

    nc = bacc.Bacc(None, target_bir_lowering=False, debug=False)
    statesT = nc.declare_dram_parameter("statesT", [d_, R], BF16, isOutput=False)
    wq = nc.declare_dram_parameter("wq", [d_, HPC], BF16, isOutput=False)
    wk = nc.declare_dram_parameter("wk", [d_, HPC], BF16, isOutput=False)
    wv = nc.declare_dram_parameter("wv", [d_, HPC], BF16, isOutput=False)
    wo = nc.declare_dram_parameter("wo", [d_, d_], BF16, isOutput=False)
    tri_in = nc.declare_dram_parameter("tri", [KJ, KJ], BF16, isOutput=False)
    out_ext = nc.declare_dram_parameter("out", [Rc, d_], F32, isOutput=True)

    SC = float(1.0 / np.sqrt(HD))
    EXP = mybir.ActivationFunctionType.Exp

    with tile.TileContext(nc) as tc:
        with tc.tile_pool(name="persist", bufs=1) as pp, \
             tc.tile_pool(name="dram", bufs=1, space="DRAM") as dram:
            # one AllToAll per half: chunks are [HPC, HR] blocks per dest core
            a2a_in = [dram.tile([n_cores * HPC, HR], BF16, tag=f"a2a_in{i}",
                                name=f"a2a_in{i}")
                      for i in range(n_half)]
            a2a_out = [dram.tile([n_cores * HPC, HR], BF16, tag=f"a2a_out{i}",
                                 name=f"a2a_out{i}")
                       for i in range(n_half)]

            qT = pp.tile([HPC, R], BF16, tag="qT")
            kT = pp.tile([HPC, R], BF16, tag="kT")
            vp = pp.tile([KJ, b_ * NH * SKJ, HD + 1], BF16, tag="vp")
            w_sb = pp.tile([128, 3, DT, HPC], BF16, tag="w_sb")
            wo_sb = pp.tile([128, DT, d_], BF16, tag="wo_sb")
            tri_sb = pp.tile([KJ, KJ], BF16, tag="tri_sb")
            ident = pp.tile([128, 128], BF16, tag="ident")

            with tc.tile_pool(name="st_in", bufs=3) as stp, \
                 tc.tile_pool(name="vT_pool", bufs=2) as vtp, \
                 tc.tile_pool(name="ps_ps", bufs=2, space="PSUM") as qps, \
                 tc.tile_pool(name="sp_ps", bufs=2, space="PSUM") as spp, \
                 tc.tile_pool(name="ctx_ps", bufs=2, space="PSUM") as cps, \
                 tc.tile_pool(name="pt_sb", bufs=6) as ptp, \
                 tc.tile_pool(name="ctxu_sb", bufs=4) as cup, \
                 tc.tile_pool(name="recip_sb", bufs=2) as rpp, \
                 tc.tile_pool(name="ctxT_sb", bufs=4) as ctp, \
                 tc.tile_pool(name="slab_sb", bufs=2) as slp, \
                 tc.tile_pool(name="o_sb", bufs=3) as osp:

                # ---- prologue: start input DMAs early, PE warms on identity
                st_tiles = {}

                def issue_st(ci):
                    st = stp.tile([128, DT, QI], BF16, tag="st", name="st")
                    for dd in range(DT):
                        nc.sync.dma_start(
                            out=st[:, dd],
                            in_=statesT[dd * 128:(dd + 1) * 128,
                                        ci * QI:(ci + 1) * QI])
                    st_tiles[ci] = st

                issue_st(0)
                issue_st(1)
                nc.sync.dma_start(out=tri_sb[:], in_=tri_in[:, :])
                for i, w in enumerate([wq, wk, wv]):
                    nc.sync.dma_start(
                        out=w_sb[:, i], in_=w[:, :].rearrange("(t p) c -> p t c", p=128))
                nc.sync.dma_start(
                    out=wo_sb[:], in_=wo[:, :].rearrange("(t p) n -> p t n", p=128))
                make_identity(nc, ident[:])
                nc.vector.memset(vp[:, :, HD], 1.0)

                def qkv_units(bb):
                    """Yield-granular QKV + V' transposes for batch bb."""
                    vT = vtp.tile([HPC, s_], BF16, tag="vT", name="vT")
                    for cl in range(CL):
                        ci = bb * CL + cl
                        if ci + 2 < b_ * CL:
                            issue_st(ci + 2)
                        st = st_tiles.pop(ci)
                        yield
                        for pi, dest, off in ((2, vT, cl * QI), (0, qT, ci * QI),
                                              (1, kT, ci * QI)):
                            ps = qps.tile([128, QI], F32, tag="ps", name="ps")
                            for dd in range(DT):
                                nc.tensor.matmul(
                                    ps[:], w_sb[:, pi, dd], st[:, dd],
                                    start=(dd == 0), stop=(dd == DT - 1))
                                if dd % 2 == 1:
                                    yield
                            nc.vector.tensor_copy(dest[:, off:off + QI], ps[:])
                            yield
                        for h in range(NH):
                            for kj in range(cl * (SKJ // CL), (cl + 1) * (SKJ // CL)):
                                blk = (bb * NH + h) * SKJ + kj
                                tp = qps.tile([KJ, HD], BF16, tag="ps", name="tp")
                                nc.tensor.transpose(
                                    tp[0:KJ, 0:HD],
                                    vT[h * HD:(h + 1) * HD, kj * KJ:(kj + 1) * KJ],
                                    ident[h * HD:(h + 1) * HD, h * HD:(h + 1) * HD])
                                nc.vector.tensor_copy(vp[:, blk, 0:HD],
                                                      tp[0:KJ, 0:HD])
                                yield

                def proj_units(hf):
                    """Yield-granular output projection for half hf."""
                    slab = slp.tile([HPC, n_cores, HR], BF16, tag="slab",
                                    name="slab")
                    for c in range(n_cores):
                        nc.sync.dma_start(
                            out=slab[:, c],
                            in_=a2a_out[hf][c * HPC:(c + 1) * HPC, :])
                    for _ in range(20):
                        yield
                    for m in range(HR // 128):
                        for n in range(d_ // QI):
                            ps = qps.tile([128, QI], F32, tag="ps", name="ops")
                            for c in range(n_cores):
                                nc.tensor.matmul(
                                    ps[:],
                                    slab[:, c, m * 128:(m + 1) * 128],
                                    wo_sb[:, c, n * QI:(n + 1) * QI],
                                    start=(c == 0), stop=(c == n_cores - 1))
                                if c % 2 == 1:
                                    yield
                            ob = osp.tile([128, QI], F32, tag="ob", name="ob")
                            nc.vector.tensor_copy(ob[:], ps[:])
                            nc.sync.dma_start(
                                out=out_ext[hf * HR + m * 128:
                                            hf * HR + (m + 1) * 128,
                                            n * QI:(n + 1) * QI],
                                in_=ob[:])
                            yield

                # Two-stage epilogue pipeline, serviced once per qi at a
                # point where the DVE queue is otherwise idle. Stage A frees
                # the ctx PSUM bank (copy incl. den row to SBUF) and runs the
                # slow DVE reciprocal + GpSimd broadcast; stage B (a full qi
                # later, so the broadcast has completed) normalizes and DMAs
                # into the AllToAll buffer.
                stage_a = []
                stage_b = []

                def service():
                    while stage_b:
                        ctxu, rb, ai, j, hh = stage_b.pop(0)
                        ctxT = ctp.tile([HD, QI], BF16, tag="ctxT",
                                        name="ctxT")
                        nc.vector.tensor_mul(ctxT[:], ctxu[0:HD, :], rb[:])
                        nc.sync.dma_start(
                            out=a2a_in[ai][j * HPC + hh * HD:
                                           j * HPC + (hh + 1) * HD, 0:QI],
                            in_=ctxT[:])
                    while stage_a:
                        ctx, ai, j, hh = stage_a.pop(0)
                        ctxu = cup.tile([HD + 1, QI], BF16, tag="ctxu",
                                        name="ctxu", bufs=4)
                        nc.vector.tensor_copy(ctxu[:], ctx[:])
                        rr = rpp.tile([1, QI], BF16, tag="recip", name="rr",
                                      bufs=4)
                        with nc.allow_low_precision(
                                reason="softmax denom reciprocal to bf16"):
                            nc.vector.reciprocal(rr[:], ctxu[HD:HD + 1, :])
                        rb = rpp.tile([HD, QI], BF16, tag="rb", name="rb",
                                      bufs=4)
                        nc.gpsimd.partition_broadcast(rb[:], rr[:])
                        stage_b.append((ctxu, rb, ai, j, hh))

                def flush_pending():
                    service()
                    service()

                def attn_bh(bb, h, fill):
                    base = bb * s_
                    hf = bb // BH
                    for qi in range(SQI):
                        ctx = cps.tile([HD + 1, QI], F32, tag="ctx", name="ctx")
                        q0 = base + qi * QI

                        def s_mm(out_ap, kj, coff):
                            nc.tensor.matmul(
                                out_ap,
                                kT[h * HD:(h + 1) * HD,
                                   base + kj * KJ: base + (kj + 1) * KJ],
                                qT[h * HD:(h + 1) * HD, q0 + coff: q0 + QI],
                                start=True, stop=True)

                        def pv_mm(kj, rhs_ap, coff, start, stop):
                            blk = (bb * NH + h) * SKJ + kj
                            nc.tensor.matmul(
                                ctx[:, coff:QI], vp[:, blk], rhs_ap,
                                start=start, stop=stop)

                        # diagonal blocks first (covers ctx fully via di=0),
                        # packed two per PSUM region: [di0|di1], [di2|di3]
                        for g in range(2):
                            di0, di1 = 2 * g, 2 * g + 1
                            n0, n1 = QI - KJ * di0, QI - KJ * di1
                            reg = spp.tile([128, 2 * QI], F32, tag="sp",
                                           name="reg")
                            s_mm(reg[:, 0:n0], SQI * qi + di0, KJ * di0)
                            s_mm(reg[:, n0:n0 + n1], SQI * qi + di1, KJ * di1)
                            pt = ptp.tile([128, 2 * QI], BF16, tag="pt",
                                          name="pt")
                            nc.scalar.activation(
                                pt[:, 0:n0 + n1], reg[:, 0:n0 + n1], EXP,
                                scale=SC)
                            nc.vector.tensor_mul(
                                pt[:, 0:KJ], pt[:, 0:KJ], tri_sb[:])
                            nc.vector.tensor_mul(
                                pt[:, n0:n0 + KJ], pt[:, n0:n0 + KJ], tri_sb[:])
                            fill(2)
                            pv_mm(SQI * qi + di0, pt[:, 0:n0], KJ * di0,
                                  start=(g == 0), stop=False)
                            pv_mm(SQI * qi + di1, pt[:, n0:n0 + n1], KJ * di1,
                                  start=False, stop=(g == 1 and qi == 0))
                            fill(1)
                        # epilogue pipeline service point: the diag mask-muls
                        # above are already queued, so the slow DVE work here
                        # cannot delay this qi's PV matmuls
                        service()
                        # full blocks, paired
                        for kjp in range(2 * qi):
                            kja, kjb = 2 * kjp, 2 * kjp + 1
                            reg = spp.tile([128, 2 * QI], F32, tag="sp",
                                           name="reg")
                            s_mm(reg[:, 0:QI], kja, 0)
                            s_mm(reg[:, QI:2 * QI], kjb, 0)
                            pt = ptp.tile([128, 2 * QI], BF16, tag="pt",
                                          name="pt")
                            nc.scalar.activation(pt[:], reg[:], EXP, scale=SC)
                            fill(2)
                            pv_mm(kja, pt[:, 0:QI], 0, start=False, stop=False)
                            pv_mm(kjb, pt[:, QI:2 * QI], 0,
                                  start=False, stop=(kjp == 2 * qi - 1))
                            fill(1)
                        stage_a.append((ctx, hf,
                                        ((base + qi * QI) % RH) // HR, h))
                        fill(1)

                def drain(gen):
                    for _ in gen:
                        pass

                def make_fill(gen):
                    box = {"g": gen}

                    def fill(n=1):
                        g = box["g"]
                        if g is None:
                            return
                        for _ in range(n):
                            try:
                                next(g)
                            except StopIteration:
                                box["g"] = None
                                return
                    return fill, box

                drain(qkv_units(0))
                for bb in range(b_):
                    if bb < b_ - 1:
                        gen = qkv_units(bb + 1)
                    else:
                        gen = proj_units(0)
                    fill, box = make_fill(gen)
                    for h in range(NH):
                        attn_bh(bb, h, fill)
                    if bb % BH == BH - 1:
                        flush_pending()
                        i = bb // BH
                        nc.gpsimd.collective_compute(
                            "AllToAll", mybir.AluOpType.bypass,
                            replica_groups=[list(range(n_cores))],
                            ins=[a2a_in[i][:].opt()],
                            outs=[a2a_out[i][:].opt()])
                    if box["g"] is not None:
                        drain(box["g"])
                drain(proj_units(1))
    nc.finalize()
    return nc


def make_in_maps(states, Wq, Wk, Wv, Wo, n_cores=N_CORES):
    b_, s_, d_ = states.shape
    R = b_ * s_
    HPC = d_ // n_cores
    bf = ml_dtypes.bfloat16
    statesT = np.ascontiguousarray(
        np.asarray(states, dtype=np.float32).reshape(R, d_).T).astype(bf)
    Wq = np.asarray(Wq, dtype=np.float32).astype(bf)
    Wk = np.asarray(Wk, dtype=np.float32).astype(bf)
    Wv = np.asarray(Wv, dtype=np.float32).astype(bf)
    Wo = np.ascontiguousarray(np.asarray(Wo, dtype=np.float32)).astype(bf)
    tri = build_tri()
    in_maps = []
    for c in range(n_cores):
        in_maps.append({
            "statesT": statesT,
            "wq": np.ascontiguousarray(Wq[:, c * HPC:(c + 1) * HPC]),
            "wk": np.ascontiguousarray(Wk[:, c * HPC:(c + 1) * HPC]),
            "wv": np.ascontiguousarray(Wv[:, c * HPC:(c + 1) * HPC]),
            "wo": Wo,
            "tri": tri,
        })
    return in_maps


def unshard(outs, b_, s_, d_, n_cores=N_CORES):
    """Core j's output rows are [half0: rows HR*j ...][half1: ...]."""
    R = b_ * s_
    Rc = R // n_cores
    n_half = 2
    HR = Rc // n_half
    RH = R // n_half
    full = np.empty((R, d_), dtype=np.float32)
    for j in range(n_cores):
        for hf in range(n_half):
            full[hf * RH + j * HR: hf * RH + (j + 1) * HR] = \
                outs[j][hf * HR:(hf + 1) * HR]
    return full.reshape(b_, s_, d_)


_NC_CACHE = {}


def kernel(states, mask, Wq, Wk, Wv, Wo):
    """Full inputs -> full output [B, S, D]. mask is causal by construction
    (reference builds tril); causality is hardcoded on-chip."""
    from concourse.bass_utils import run_bass_kernel_spmd

    states = np.asarray(states, dtype=np.float32)
    b_, s_, d_ = states.shape
    key = (b_, s_, d_)
    if key not in _NC_CACHE:
        _NC_CACHE[key] = build(b_, s_, d_)
    nc = _NC_CACHE[key]
    in_maps = make_in_maps(states, Wq, Wk, Wv, Wo)
    res = run_bass_kernel_spmd(nc, in_maps, core_ids=list(range(N_CORES)))
    outs = [res.results[c]["out"] for c in range(N_CORES)]
    return unshard(outs, b_, s_, d_).astype(np.float32)


# revision 37
# speedup vs baseline: 1.0764x; 1.0764x over previous
"""Distributed attention layer kernel for 8 TRN2 NeuronCores.

Math (per reference): out = softmax_causal((x@Wq)(x@Wk)^T / 8) @ (x@Wv) @ Wo
with B=4, S=2048, D=1024, H=16 heads of dim 64.

Sharding: head tensor-parallel. Core c owns head pair (2c, 2c+1):
  - Wq/Wk/Wv column-sharded [1024, 128]; statesT replicated [1024, 8192].
  - Each core computes qT/kT/vT for its 2 heads, causal attention in
    S^T layout (kj on partitions, qi on free), softmax denominator via a
    ones-column appended to V (PV matmul row 64 = sum of probs).
  - ctx^T tiles are normalized on the fly (reciprocal straight from the
    PSUM denominator row, partition_broadcast on GpSimd, one DVE mul)
    and staged into per-half AllToAll buffers laid out so chunk j =
    [128 head cols, rows of output-core j].
  - Output projection: out_rows = sum_c slab_c.T @ Wo[128c:...] in PSUM.

Scheduling: the PE instruction stream is kept dense to hold the clock
at the top p-state. QKV for batch b+1 is interleaved as filler between
the score/PV matmul pairs of batch b's attention (v-projection first,
so its PSUM->SBUF copy clears the in-order DVE queue long before the
V' transposes need it); the half-0 output projection is interleaved
into batch 3's attention so AllToAll #0 (which also absorbs the
inter-core start skew once) overlaps ~200us of compute. One AllToAll
per half - split collectives serialize on the cc stream and cascade
under start skew. Causal masking of diagonal blocks is a post-exp
multiply by a 0/1 triangular mask (bf16, SBUF) so the score->exp chain
never waits on a PSUM-side DVE add.

Matmul operands are bf16 (PE full rate); accumulation is fp32 in PSUM.
"""

import ml_dtypes
import numpy as np

import concourse.bass as bass
import concourse.mybir as mybir
import concourse.tile as tile
from concourse import bacc
from concourse.masks import make_identity

F32 = mybir.dt.float32
BF16 = mybir.dt.bfloat16

B, S, D, H = 4, 2048, 1024, 16
HD = 64
N_CORES = 8
QI, KJ = 512, 128


def build_tri(KJ=KJ):
    """tri[p, f] = 1.0 if p <= f else 0.0 (valid causal positions of the
    first KJ columns of a diagonal strip)."""
    p = np.arange(KJ)[:, None]
    f = np.arange(KJ)[None, :]
    return np.where(p <= f, 1.0, 0.0).astype(ml_dtypes.bfloat16)


def build(b_=B, s_=S, d_=D, n_cores=N_CORES):
    HPC = d_ // n_cores          # head cols per core (2 heads x 64)
    NH = HPC // HD               # heads per core (2)
    R = b_ * s_                  # global rows (8192)
    Rc = R // n_cores            # output rows per core (1024)
    DT = d_ // 128               # contraction tiles (8)
    SKJ = s_ // KJ               # kj blocks per (b, h) (16)
    SQI = s_ // QI               # qi tiles per (b, h) (4)
    n_half = 2
    BH = b_ // n_half            # batches per half (2)
    HR = Rc // n_half            # rows per core per half (512)
    RH = R // n_half             # global rows per half (4096)
    CL = s_ // QI                # column tiles per batch (4)
    assert s_ % QI == 0 and HR == QI and d_ % 128 == 0

    nc = bacc.Bacc(None, target_bir_lowering=False, debug=False)
    statesT = nc.declare_dram_parameter("statesT", [d_, R], BF16, isOutput=False)
    wq = nc.declare_dram_parameter("wq", [d_, HPC], BF16, isOutput=False)
    wk = nc.declare_dram_parameter("wk", [d_, HPC], BF16, isOutput=False)
    wv = nc.declare_dram_parameter("wv", [d_, HPC], BF16, isOutput=False)
    wo = nc.declare_dram_parameter("wo", [d_, d_], BF16, isOutput=False)
    tri_in = nc.declare_dram_parameter("tri", [KJ, KJ], BF16, isOutput=False)
    out_ext = nc.declare_dram_parameter("out", [Rc, d_], F32, isOutput=True)

    SC = float(1.0 / np.sqrt(HD))
    EXP = mybir.ActivationFunctionType.Exp

    with tile.TileContext(nc) as tc:
        with tc.tile_pool(name="persist", bufs=1) as pp, \
             tc.tile_pool(name="dram", bufs=1, space="DRAM") as dram:
            # one AllToAll per half: chunks are [HPC, HR] blocks per dest core
            a2a_in = [dram.tile([n_cores * HPC, HR], BF16, tag=f"a2a_in{i}",
                                name=f"a2a_in{i}")
                      for i in range(n_half)]
            a2a_out = [dram.tile([n_cores * HPC, HR], BF16, tag=f"a2a_out{i}",
                                 name=f"a2a_out{i}")
                       for i in range(n_half)]

            qT = pp.tile([HPC, R], BF16, tag="qT")
            kT = pp.tile([HPC, R], BF16, tag="kT")
            vp = pp.tile([KJ, b_ * NH * SKJ, HD + 1], BF16, tag="vp")
            w_sb = pp.tile([128, 3, DT, HPC], BF16, tag="w_sb")
            wo_sb = pp.tile([128, DT, d_], BF16, tag="wo_sb")
            tri_sb = pp.tile([KJ, KJ], BF16, tag="tri_sb")
            ident = pp.tile([128, 128], BF16, tag="ident")

            with tc.tile_pool(name="st_in", bufs=3) as stp, \
                 tc.tile_pool(name="vT_pool", bufs=2) as vtp, \
                 tc.tile_pool(name="ps_ps", bufs=2, space="PSUM") as qps, \
                 tc.tile_pool(name="sp_ps", bufs=2, space="PSUM") as spp, \
                 tc.tile_pool(name="ctx_ps", bufs=2, space="PSUM") as cps, \
                 tc.tile_pool(name="pt_sb", bufs=6) as ptp, \
                 tc.tile_pool(name="ctxu_sb", bufs=4) as cup, \
                 tc.tile_pool(name="recip_sb", bufs=2) as rpp, \
                 tc.tile_pool(name="ctxT_sb", bufs=4) as ctp, \
                 tc.tile_pool(name="slab_sb", bufs=2) as slp, \
                 tc.tile_pool(name="o_sb", bufs=3) as osp:

                # ---- prologue: start input DMAs early, PE warms on identity
                st_tiles = {}

                def issue_st(ci):
                    st = stp.tile([128, DT, QI], BF16, tag="st", name="st")
                    for dd in range(DT):
                        nc.sync.dma_start(
                            out=st[:, dd],
                            in_=statesT[dd * 128:(dd + 1) * 128,
                                        ci * QI:(ci + 1) * QI])
                    st_tiles[ci] = st

                issue_st(0)
                issue_st(1)
                nc.sync.dma_start(out=tri_sb[:], in_=tri_in[:, :])
                for i, w in enumerate([wq, wk, wv]):
                    nc.sync.dma_start(
                        out=w_sb[:, i], in_=w[:, :].rearrange("(t p) c -> p t c", p=128))
                nc.sync.dma_start(
                    out=wo_sb[:], in_=wo[:, :].rearrange("(t p) n -> p t n", p=128))
                make_identity(nc, ident[:])
                nc.vector.memset(vp[:, :, HD], 1.0)

                def qkv_units(bb):
                    """Yield-granular QKV + V' transposes for batch bb."""
                    vT = vtp.tile([HPC, s_], BF16, tag="vT", name="vT")
                    for cl in range(CL):
                        ci = bb * CL + cl
                        if ci + 2 < b_ * CL:
                            issue_st(ci + 2)
                        st = st_tiles.pop(ci)
                        yield
                        for pi, dest, off in ((2, vT, cl * QI), (0, qT, ci * QI),
                                              (1, kT, ci * QI)):
                            ps = qps.tile([128, QI], F32, tag="ps", name="ps")
                            for dd in range(DT):
                                nc.tensor.matmul(
                                    ps[:], w_sb[:, pi, dd], st[:, dd],
                                    start=(dd == 0), stop=(dd == DT - 1))
                                if dd % 2 == 1:
                                    yield
                            nc.vector.tensor_copy(dest[:, off:off + QI], ps[:])
                            yield
                        for h in range(NH):
                            for kj in range(cl * (SKJ // CL), (cl + 1) * (SKJ // CL)):
                                blk = (bb * NH + h) * SKJ + kj
                                tp = qps.tile([KJ, HD], BF16, tag="ps", name="tp")
                                nc.tensor.transpose(
                                    tp[0:KJ, 0:HD],
                                    vT[h * HD:(h + 1) * HD, kj * KJ:(kj + 1) * KJ],
                                    ident[h * HD:(h + 1) * HD, h * HD:(h + 1) * HD])
                                nc.vector.tensor_copy(vp[:, blk, 0:HD],
                                                      tp[0:KJ, 0:HD])
                                yield

                def proj_units(hf):
                    """Yield-granular output projection for half hf."""
                    slab = slp.tile([HPC, n_cores, HR], BF16, tag="slab",
                                    name="slab")
                    for c in range(n_cores):
                        nc.sync.dma_start(
                            out=slab[:, c],
                            in_=a2a_out[hf][c * HPC:(c + 1) * HPC, :])
                    for _ in range(20):
                        yield
                    for m in range(HR // 128):
                        for n in range(d_ // QI):
                            ps = qps.tile([128, QI], F32, tag="ps", name="ops")
                            for c in range(n_cores):
                                nc.tensor.matmul(
                                    ps[:],
                                    slab[:, c, m * 128:(m + 1) * 128],
                                    wo_sb[:, c, n * QI:(n + 1) * QI],
                                    start=(c == 0), stop=(c == n_cores - 1))
                                if c % 2 == 1:
                                    yield
                            ob = osp.tile([128, QI], F32, tag="ob", name="ob")
                            nc.vector.tensor_copy(ob[:], ps[:])
                            nc.sync.dma_start(
                                out=out_ext[hf * HR + m * 128:
                                            hf * HR + (m + 1) * 128,
                                            n * QI:(n + 1) * QI],
                                in_=ob[:])
                            yield

                # Two-stage epilogue pipeline, serviced once per qi at a
                # point where the DVE queue is otherwise idle. Stage A frees
                # the ctx PSUM bank (copy incl. den row to SBUF) and runs the
                # slow DVE reciprocal + GpSimd broadcast; stage B (a full qi
                # later, so the broadcast has completed) normalizes and DMAs
                # into the AllToAll buffer.
                stage_a = []
                stage_b = []
                side = []     # small deferred DVE ops (reciprocal slices)
                NSL = 8       # reciprocal slices per qi tile

                def drain_side(n):
                    for _ in range(min(n, len(side))):
                        side.pop(0)()

                def service():
                    # all pending reciprocal slices must be emitted before a
                    # stage-B mul that reads them (program order = dep order)
                    if stage_b:
                        drain_side(len(side))
                    while stage_b:
                        ctxu, rb, ai, j, hh = stage_b.pop(0)
                        ctxT = ctp.tile([HD, QI], BF16, tag="ctxT",
                                        name="ctxT")
                        nc.vector.tensor_mul(ctxT[:], ctxu[0:HD, :], rb[:])
                        nc.sync.dma_start(
                            out=a2a_in[ai][j * HPC + hh * HD:
                                           j * HPC + (hh + 1) * HD, 0:QI],
                            in_=ctxT[:])
                    while stage_a:
                        ctx, ai, j, hh = stage_a.pop(0)
                        # den must land on partition 0: partition_broadcast
                        # broadcasts partition 0 regardless of the AP offset
                        den0 = rpp.tile([1, QI], BF16, tag="den0",
                                        name="den0", bufs=4)
                        nc.vector.tensor_copy(den0[:], ctx[HD:HD + 1, :])
                        ctxu = cup.tile([HD, QI], BF16, tag="ctxu",
                                        name="ctxu", bufs=4)
                        nc.vector.tensor_copy(ctxu[:], ctx[0:HD, :])
                        # broadcast the raw denominator row, then invert in
                        # small preemptible slices so the in-order DVE queue
                        # never blocks the mask-muls behind a 4us reciprocal
                        rbd = rpp.tile([HD, QI], BF16, tag="rbd", name="rbd",
                                       bufs=4)
                        nc.gpsimd.partition_broadcast(rbd[:], den0[:])
                        rb = rpp.tile([HD, QI], BF16, tag="rb", name="rb",
                                      bufs=4)
                        SL = QI // NSL

                        def mk(s, rb=rb, rbd=rbd):
                            def emit():
                                with nc.allow_low_precision(
                                        reason="softmax recip to bf16"):
                                    nc.vector.reciprocal(
                                        rb[:, s * SL:(s + 1) * SL],
                                        rbd[:, s * SL:(s + 1) * SL])
                            return emit

                        for s in range(NSL):
                            side.append(mk(s))
                        stage_b.append((ctxu, rb, ai, j, hh))

                def flush_pending():
                    drain_side(len(side))
                    service()
                    drain_side(len(side))
                    service()

                def attn_bh(bb, h, fill):
                    base = bb * s_
                    hf = bb // BH
                    for qi in range(SQI):
                        ctx = cps.tile([HD + 1, QI], F32, tag="ctx", name="ctx")
                        q0 = base + qi * QI

                        def s_mm(out_ap, kj, coff):
                            nc.tensor.matmul(
                                out_ap,
                                kT[h * HD:(h + 1) * HD,
                                   base + kj * KJ: base + (kj + 1) * KJ],
                                qT[h * HD:(h + 1) * HD, q0 + coff: q0 + QI],
                                start=True, stop=True)

                        def pv_mm(kj, rhs_ap, coff, start, stop):
                            blk = (bb * NH + h) * SKJ + kj
                            nc.tensor.matmul(
                                ctx[:, coff:QI], vp[:, blk], rhs_ap,
                                start=start, stop=stop)

                        # diagonal blocks first (covers ctx fully via di=0),
                        # packed two per PSUM region: [di0|di1], [di2|di3]
                        for g in range(2):
                            di0, di1 = 2 * g, 2 * g + 1
                            n0, n1 = QI - KJ * di0, QI - KJ * di1
                            reg = spp.tile([128, 2 * QI], F32, tag="sp",
                                           name="reg")
                            s_mm(reg[:, 0:n0], SQI * qi + di0, KJ * di0)
                            s_mm(reg[:, n0:n0 + n1], SQI * qi + di1, KJ * di1)
                            pt = ptp.tile([128, 2 * QI], BF16, tag="pt",
                                          name="pt")
                            nc.scalar.activation(
                                pt[:, 0:n0 + n1], reg[:, 0:n0 + n1], EXP,
                                scale=SC)
                            nc.vector.tensor_mul(
                                pt[:, 0:KJ], pt[:, 0:KJ], tri_sb[:])
                            nc.vector.tensor_mul(
                                pt[:, n0:n0 + KJ], pt[:, n0:n0 + KJ], tri_sb[:])
                            fill(2)
                            pv_mm(SQI * qi + di0, pt[:, 0:n0], KJ * di0,
                                  start=(g == 0), stop=False)
                            pv_mm(SQI * qi + di1, pt[:, n0:n0 + n1], KJ * di1,
                                  start=False, stop=(g == 1 and qi == 0))
                            fill(1)
                            drain_side(2)
                        # epilogue pipeline service point: the diag mask-muls
                        # above are already queued, so the slow DVE work here
                        # cannot delay this qi's PV matmuls
                        service()
                        # full blocks, paired
                        for kjp in range(2 * qi):
                            kja, kjb = 2 * kjp, 2 * kjp + 1
                            reg = spp.tile([128, 2 * QI], F32, tag="sp",
                                           name="reg")
                            s_mm(reg[:, 0:QI], kja, 0)
                            s_mm(reg[:, QI:2 * QI], kjb, 0)
                            pt = ptp.tile([128, 2 * QI], BF16, tag="pt",
                                          name="pt")
                            nc.scalar.activation(pt[:], reg[:], EXP, scale=SC)
                            fill(2)
                            pv_mm(kja, pt[:, 0:QI], 0, start=False, stop=False)
                            pv_mm(kjb, pt[:, QI:2 * QI], 0,
                                  start=False, stop=(kjp == 2 * qi - 1))
                            fill(1)
                            drain_side(2)
                        stage_a.append((ctx, hf,
                                        ((base + qi * QI) % RH) // HR, h))
                        fill(1)

                def drain(gen):
                    for _ in gen:
                        pass

                def make_fill(gen):
                    box = {"g": gen}

                    def fill(n=1):
                        g = box["g"]
                        if g is None:
                            return
                        for _ in range(n):
                            try:
                                next(g)
                            except StopIteration:
                                box["g"] = None
                                return
                    return fill, box

                drain(qkv_units(0))
                for bb in range(b_):
                    if bb < b_ - 1:
                        gen = qkv_units(bb + 1)
                    else:
                        gen = proj_units(0)
                    fill, box = make_fill(gen)
                    for h in range(NH):
                        attn_bh(bb, h, fill)
                    if bb % BH == BH - 1:
                        flush_pending()
                        i = bb // BH
                        nc.gpsimd.collective_compute(
                            "AllToAll", mybir.AluOpType.bypass,
                            replica_groups=[list(range(n_cores))],
                            ins=[a2a_in[i][:].opt()],
                            outs=[a2a_out[i][:].opt()])
                    if box["g"] is not None:
                        drain(box["g"])
                drain(proj_units(1))
    nc.finalize()
    return nc


def make_in_maps(states, Wq, Wk, Wv, Wo, n_cores=N_CORES):
    b_, s_, d_ = states.shape
    R = b_ * s_
    HPC = d_ // n_cores
    bf = ml_dtypes.bfloat16
    statesT = np.ascontiguousarray(
        np.asarray(states, dtype=np.float32).reshape(R, d_).T).astype(bf)
    Wq = np.asarray(Wq, dtype=np.float32).astype(bf)
    Wk = np.asarray(Wk, dtype=np.float32).astype(bf)
    Wv = np.asarray(Wv, dtype=np.float32).astype(bf)
    Wo = np.ascontiguousarray(np.asarray(Wo, dtype=np.float32)).astype(bf)
    tri = build_tri()
    in_maps = []
    for c in range(n_cores):
        in_maps.append({
            "statesT": statesT,
            "wq": np.ascontiguousarray(Wq[:, c * HPC:(c + 1) * HPC]),
            "wk": np.ascontiguousarray(Wk[:, c * HPC:(c + 1) * HPC]),
            "wv": np.ascontiguousarray(Wv[:, c * HPC:(c + 1) * HPC]),
            "wo": Wo,
            "tri": tri,
        })
    return in_maps


def unshard(outs, b_, s_, d_, n_cores=N_CORES):
    """Core j's output rows are [half0: rows HR*j ...][half1: ...]."""
    R = b_ * s_
    Rc = R // n_cores
    n_half = 2
    HR = Rc // n_half
    RH = R // n_half
    full = np.empty((R, d_), dtype=np.float32)
    for j in range(n_cores):
        for hf in range(n_half):
            full[hf * RH + j * HR: hf * RH + (j + 1) * HR] = \
                outs[j][hf * HR:(hf + 1) * HR]
    return full.reshape(b_, s_, d_)


_NC_CACHE = {}


def kernel(states, mask, Wq, Wk, Wv, Wo):
    """Full inputs -> full output [B, S, D]. mask is causal by construction
    (reference builds tril); causality is hardcoded on-chip."""
    from concourse.bass_utils import run_bass_kernel_spmd

    states = np.asarray(states, dtype=np.float32)
    b_, s_, d_ = states.shape
    key = (b_, s_, d_)
    if key not in _NC_CACHE:
        _NC_CACHE[key] = build(b_, s_, d_)
    nc = _NC_CACHE[key]
    in_maps = make_in_maps(states, Wq, Wk, Wv, Wo)
    res = run_bass_kernel_spmd(nc, in_maps, core_ids=list(range(N_CORES)))
    outs = [res.results[c]["out"] for c in range(N_CORES)]
    return unshard(outs, b_, s_, d_).astype(np.float32)


# revision 40
# speedup vs baseline: 1.1176x; 1.0382x over previous
"""Distributed attention layer kernel for 8 TRN2 NeuronCores.

Math (per reference): out = softmax_causal((x@Wq)(x@Wk)^T / 8) @ (x@Wv) @ Wo
with B=4, S=2048, D=1024, H=16 heads of dim 64.

Sharding: head tensor-parallel. Core c owns head pair (2c, 2c+1):
  - Wq/Wk/Wv column-sharded [1024, 128]; statesT replicated [1024, 8192].
  - Each core computes qT/kT/vT for its 2 heads, causal attention in
    S^T layout (kj on partitions, qi on free), softmax denominator via a
    ones-column appended to V (PV matmul row 64 = sum of probs).
  - ctx^T tiles are normalized on the fly (reciprocal straight from the
    PSUM denominator row, partition_broadcast on GpSimd, one DVE mul)
    and staged into per-half AllToAll buffers laid out so chunk j =
    [128 head cols, rows of output-core j].
  - Output projection: out_rows = sum_c slab_c.T @ Wo[128c:...] in PSUM.

Scheduling: the PE instruction stream is kept dense to hold the clock
at the top p-state. QKV for batch b+1 is interleaved as filler between
the score/PV matmul pairs of batch b's attention (v-projection first,
so its PSUM->SBUF copy clears the in-order DVE queue long before the
V' transposes need it); the half-0 output projection is interleaved
into batch 3's attention so AllToAll #0 (which also absorbs the
inter-core start skew once) overlaps ~200us of compute. One AllToAll
per half - split collectives serialize on the cc stream and cascade
under start skew. Causal masking of diagonal blocks is a post-exp
multiply by a 0/1 triangular mask (bf16, SBUF) so the score->exp chain
never waits on a PSUM-side DVE add.

Matmul operands are bf16 (PE full rate); accumulation is fp32 in PSUM.
"""

import ml_dtypes
import numpy as np

import concourse.bass as bass
import concourse.mybir as mybir
import concourse.tile as tile
from concourse import bacc
from concourse.masks import make_identity

F32 = mybir.dt.float32
BF16 = mybir.dt.bfloat16

B, S, D, H = 4, 2048, 1024, 16
HD = 64
N_CORES = 8
QI, KJ = 512, 128


def build_tri(KJ=KJ):
    """tri[p, f] = 1.0 if p <= f else 0.0 (valid causal positions of the
    first KJ columns of a diagonal strip)."""
    p = np.arange(KJ)[:, None]
    f = np.arange(KJ)[None, :]
    return np.where(p <= f, 1.0, 0.0).astype(ml_dtypes.bfloat16)


def build(b_=B, s_=S, d_=D, n_cores=N_CORES):
    HPC = d_ // n_cores          # head cols per core (2 heads x 64)
    NH = HPC // HD               # heads per core (2)
    R = b_ * s_                  # global rows (8192)
    Rc = R // n_cores            # output rows per core (1024)
    DT = d_ // 128               # contraction tiles (8)
    SKJ = s_ // KJ               # kj blocks per (b, h) (16)
    SQI = s_ // QI               # qi tiles per (b, h) (4)
    n_half = 2
    BH = b_ // n_half            # batches per half (2)
    HR = Rc // n_half            # rows per core per half (512)
    RH = R // n_half             # global rows per half (4096)
    CL = s_ // QI                # column tiles per batch (4)
    assert s_ % QI == 0 and HR == QI and d_ % 128 == 0

    nc = bacc.Bacc(None, target_bir_lowering=False, debug=False)
    statesT = nc.declare_dram_parameter("statesT", [d_, R], BF16, isOutput=False)
    wq = nc.declare_dram_parameter("wq", [d_, HPC], BF16, isOutput=False)
    wk = nc.declare_dram_parameter("wk", [d_, HPC], BF16, isOutput=False)
    wv = nc.declare_dram_parameter("wv", [d_, HPC], BF16, isOutput=False)
    wo = nc.declare_dram_parameter("wo", [d_, d_], BF16, isOutput=False)
    tri_in = nc.declare_dram_parameter("tri", [KJ, KJ], BF16, isOutput=False)
    out_ext = nc.declare_dram_parameter("out", [Rc, d_], F32, isOutput=True)

    SC = float(1.0 / np.sqrt(HD))
    EXP = mybir.ActivationFunctionType.Exp

    with tile.TileContext(nc) as tc:
        with tc.tile_pool(name="persist", bufs=1) as pp, \
             tc.tile_pool(name="dram", bufs=1, space="DRAM") as dram:
            # one AllToAll per half: chunks are [HPC, HR] blocks per dest core
            a2a_in = [dram.tile([n_cores * HPC, HR], BF16, tag=f"a2a_in{i}",
                                name=f"a2a_in{i}")
                      for i in range(n_half)]
            a2a_out = [dram.tile([n_cores * HPC, HR], BF16, tag=f"a2a_out{i}",
                                 name=f"a2a_out{i}")
                       for i in range(n_half)]

            qT = pp.tile([HPC, R], BF16, tag="qT")
            kT = pp.tile([HPC, R], BF16, tag="kT")
            vp = pp.tile([KJ, b_ * NH * SKJ, HD + 1], BF16, tag="vp")
            w_sb = pp.tile([128, 3, DT, HPC], BF16, tag="w_sb")
            wo_sb = pp.tile([128, DT, d_], BF16, tag="wo_sb")
            tri_sb = pp.tile([KJ, KJ], BF16, tag="tri_sb")
            ident = pp.tile([128, 128], BF16, tag="ident")

            with tc.tile_pool(name="st_in", bufs=3) as stp, \
                 tc.tile_pool(name="vT_pool", bufs=2) as vtp, \
                 tc.tile_pool(name="ps_ps", bufs=2, space="PSUM") as qps, \
                 tc.tile_pool(name="sp_ps", bufs=2, space="PSUM") as spp, \
                 tc.tile_pool(name="ctx_ps", bufs=2, space="PSUM") as cps, \
                 tc.tile_pool(name="pt_sb", bufs=6) as ptp, \
                 tc.tile_pool(name="ctxu_sb", bufs=4) as cup, \
                 tc.tile_pool(name="recip_sb", bufs=2) as rpp, \
                 tc.tile_pool(name="ctxT_sb", bufs=4) as ctp, \
                 tc.tile_pool(name="slab_sb", bufs=2) as slp, \
                 tc.tile_pool(name="o_sb", bufs=3) as osp:

                # ---- prologue: start input DMAs early, PE warms on identity
                st_tiles = {}

                def issue_st(ci):
                    st = stp.tile([128, DT, QI], BF16, tag="st", name="st")
                    for dd in range(DT):
                        nc.sync.dma_start(
                            out=st[:, dd],
                            in_=statesT[dd * 128:(dd + 1) * 128,
                                        ci * QI:(ci + 1) * QI])
                    st_tiles[ci] = st

                issue_st(0)
                issue_st(1)
                nc.sync.dma_start(out=tri_sb[:], in_=tri_in[:, :])
                for i, w in enumerate([wq, wk, wv]):
                    nc.sync.dma_start(
                        out=w_sb[:, i], in_=w[:, :].rearrange("(t p) c -> p t c", p=128))
                nc.sync.dma_start(
                    out=wo_sb[:], in_=wo[:, :].rearrange("(t p) n -> p t n", p=128))
                make_identity(nc, ident[:])
                nc.vector.memset(vp[:, :, HD], 1.0)

                def qkv_units(bb):
                    """Yield-granular QKV + V' transposes for batch bb."""
                    vT = vtp.tile([HPC, s_], BF16, tag="vT", name="vT")
                    for cl in range(CL):
                        ci = bb * CL + cl
                        if ci + 2 < b_ * CL:
                            issue_st(ci + 2)
                        st = st_tiles.pop(ci)
                        yield
                        for pi, dest, off in ((2, vT, cl * QI), (0, qT, ci * QI),
                                              (1, kT, ci * QI)):
                            ps = qps.tile([128, QI], F32, tag="ps", name="ps")
                            for dd in range(DT):
                                nc.tensor.matmul(
                                    ps[:], w_sb[:, pi, dd], st[:, dd],
                                    start=(dd == 0), stop=(dd == DT - 1))
                                if dd % 2 == 1:
                                    yield
                            nc.vector.tensor_copy(dest[:, off:off + QI], ps[:])
                            yield
                        for h in range(NH):
                            for kj in range(cl * (SKJ // CL), (cl + 1) * (SKJ // CL)):
                                blk = (bb * NH + h) * SKJ + kj
                                tp = qps.tile([KJ, HD], BF16, tag="ps", name="tp")
                                nc.tensor.transpose(
                                    tp[0:KJ, 0:HD],
                                    vT[h * HD:(h + 1) * HD, kj * KJ:(kj + 1) * KJ],
                                    ident[h * HD:(h + 1) * HD, h * HD:(h + 1) * HD])
                                nc.vector.tensor_copy(vp[:, blk, 0:HD],
                                                      tp[0:KJ, 0:HD])
                                yield

                def proj_units(hf):
                    """Yield-granular output projection for half hf."""
                    slab = slp.tile([HPC, n_cores, HR], BF16, tag="slab",
                                    name="slab")
                    for c in range(n_cores):
                        nc.sync.dma_start(
                            out=slab[:, c],
                            in_=a2a_out[hf][c * HPC:(c + 1) * HPC, :])
                    for _ in range(20):
                        yield
                    for m in range(HR // 128):
                        for n in range(d_ // QI):
                            ps = qps.tile([128, QI], F32, tag="ps", name="ops")
                            for c in range(n_cores):
                                nc.tensor.matmul(
                                    ps[:],
                                    slab[:, c, m * 128:(m + 1) * 128],
                                    wo_sb[:, c, n * QI:(n + 1) * QI],
                                    start=(c == 0), stop=(c == n_cores - 1))
                                if c % 2 == 1:
                                    yield
                            ob = osp.tile([128, QI], F32, tag="ob", name="ob")
                            nc.vector.tensor_copy(ob[:], ps[:])
                            nc.sync.dma_start(
                                out=out_ext[hf * HR + m * 128:
                                            hf * HR + (m + 1) * 128,
                                            n * QI:(n + 1) * QI],
                                in_=ob[:])
                            yield

                # Two-stage epilogue pipeline, serviced once per qi at a
                # point where the DVE queue is otherwise idle. Stage A frees
                # the ctx PSUM bank (copy incl. den row to SBUF) and runs the
                # slow DVE reciprocal + GpSimd broadcast; stage B (a full qi
                # later, so the broadcast has completed) normalizes and DMAs
                # into the AllToAll buffer.
                stage_a = []
                stage_b = []
                side = []     # per-tile lists of deferred reciprocal slices
                NSL = 8       # reciprocal slices per qi tile

                def drain_side(n):
                    while n > 0 and side:
                        lst = side[0]
                        while lst and n > 0:
                            lst.pop(0)()
                            n -= 1
                        if not lst:
                            side.pop(0)

                def service():
                    while stage_b:
                        ctxu, rb, ai, j, hh, slices = stage_b.pop(0)
                        # emit only THIS tile's leftover slices (normally
                        # none - they were spread across earlier regions)
                        while slices:
                            slices.pop(0)()
                        if side and side[0] is slices:
                            side.pop(0)
                        ctxT = ctp.tile([HD, QI], BF16, tag="ctxT",
                                        name="ctxT")
                        nc.vector.tensor_mul(ctxT[:], ctxu[0:HD, :], rb[:])
                        nc.sync.dma_start(
                            out=a2a_in[ai][j * HPC + hh * HD:
                                           j * HPC + (hh + 1) * HD, 0:QI],
                            in_=ctxT[:])
                    while stage_a:
                        ctx, ai, j, hh = stage_a.pop(0)
                        # den must land on partition 0: partition_broadcast
                        # broadcasts partition 0 regardless of the AP offset
                        den0 = rpp.tile([1, QI], BF16, tag="den0",
                                        name="den0", bufs=4)
                        nc.vector.tensor_copy(den0[:], ctx[HD:HD + 1, :])
                        ctxu = cup.tile([HD, QI], BF16, tag="ctxu",
                                        name="ctxu", bufs=4)
                        nc.vector.tensor_copy(ctxu[:], ctx[0:HD, :])
                        # broadcast the raw denominator row, then invert in
                        # small preemptible slices so the in-order DVE queue
                        # never blocks the mask-muls behind a 4us reciprocal
                        rbd = rpp.tile([HD, QI], BF16, tag="rbd", name="rbd",
                                       bufs=4)
                        nc.gpsimd.partition_broadcast(rbd[:], den0[:])
                        rb = rpp.tile([HD, QI], BF16, tag="rb", name="rb",
                                      bufs=4)
                        SL = QI // NSL

                        def mk(s, rb=rb, rbd=rbd):
                            def emit():
                                with nc.allow_low_precision(
                                        reason="softmax recip to bf16"):
                                    nc.vector.reciprocal(
                                        rb[:, s * SL:(s + 1) * SL],
                                        rbd[:, s * SL:(s + 1) * SL])
                            return emit

                        slices = [mk(s) for s in range(NSL)]
                        side.append(slices)
                        stage_b.append((ctxu, rb, ai, j, hh, slices))

                def flush_pending():
                    drain_side(10 ** 6)
                    service()
                    drain_side(10 ** 6)
                    service()

                def attn_bh(bb, h, fill):
                    base = bb * s_
                    hf = bb // BH
                    for qi in range(SQI):
                        ctx = cps.tile([HD + 1, QI], F32, tag="ctx", name="ctx")
                        q0 = base + qi * QI

                        def s_mm(out_ap, kj, coff):
                            nc.tensor.matmul(
                                out_ap,
                                kT[h * HD:(h + 1) * HD,
                                   base + kj * KJ: base + (kj + 1) * KJ],
                                qT[h * HD:(h + 1) * HD, q0 + coff: q0 + QI],
                                start=True, stop=True)

                        def pv_mm(kj, rhs_ap, coff, start, stop):
                            blk = (bb * NH + h) * SKJ + kj
                            nc.tensor.matmul(
                                ctx[:, coff:QI], vp[:, blk], rhs_ap,
                                start=start, stop=stop)

                        # diagonal blocks first (covers ctx fully via di=0),
                        # packed two per PSUM region: [di0|di1], [di2|di3]
                        for g in range(2):
                            di0, di1 = 2 * g, 2 * g + 1
                            n0, n1 = QI - KJ * di0, QI - KJ * di1
                            reg = spp.tile([128, 2 * QI], F32, tag="sp",
                                           name="reg")
                            s_mm(reg[:, 0:n0], SQI * qi + di0, KJ * di0)
                            s_mm(reg[:, n0:n0 + n1], SQI * qi + di1, KJ * di1)
                            pt = ptp.tile([128, 2 * QI], BF16, tag="pt",
                                          name="pt")
                            nc.scalar.activation(
                                pt[:, 0:n0 + n1], reg[:, 0:n0 + n1], EXP,
                                scale=SC)
                            nc.vector.tensor_mul(
                                pt[:, 0:KJ], pt[:, 0:KJ], tri_sb[:])
                            nc.vector.tensor_mul(
                                pt[:, n0:n0 + KJ], pt[:, n0:n0 + KJ], tri_sb[:])
                            fill(2)
                            pv_mm(SQI * qi + di0, pt[:, 0:n0], KJ * di0,
                                  start=(g == 0), stop=False)
                            pv_mm(SQI * qi + di1, pt[:, n0:n0 + n1], KJ * di1,
                                  start=False, stop=(g == 1 and qi == 0))
                            fill(1)
                            drain_side(2)
                        # epilogue pipeline service point: the diag mask-muls
                        # above are already queued, so the slow DVE work here
                        # cannot delay this qi's PV matmuls
                        service()
                        # full blocks, paired
                        for kjp in range(2 * qi):
                            kja, kjb = 2 * kjp, 2 * kjp + 1
                            reg = spp.tile([128, 2 * QI], F32, tag="sp",
                                           name="reg")
                            s_mm(reg[:, 0:QI], kja, 0)
                            s_mm(reg[:, QI:2 * QI], kjb, 0)
                            pt = ptp.tile([128, 2 * QI], BF16, tag="pt",
                                          name="pt")
                            nc.scalar.activation(pt[:], reg[:], EXP, scale=SC)
                            fill(2)
                            pv_mm(kja, pt[:, 0:QI], 0, start=False, stop=False)
                            pv_mm(kjb, pt[:, QI:2 * QI], 0,
                                  start=False, stop=(kjp == 2 * qi - 1))
                            fill(1)
                            drain_side(2)
                        stage_a.append((ctx, hf,
                                        ((base + qi * QI) % RH) // HR, h))
                        fill(1)

                def drain(gen):
                    for _ in gen:
                        pass

                def make_fill(gen):
                    box = {"g": gen}

                    def fill(n=1):
                        g = box["g"]
                        if g is None:
                            return
                        for _ in range(n):
                            try:
                                next(g)
                            except StopIteration:
                                box["g"] = None
                                return
                    return fill, box

                drain(qkv_units(0))
                for bb in range(b_):
                    if bb < b_ - 1:
                        gen = qkv_units(bb + 1)
                    else:
                        gen = proj_units(0)
                    fill, box = make_fill(gen)
                    for h in range(NH):
                        attn_bh(bb, h, fill)
                    if bb % BH == BH - 1:
                        flush_pending()
                        i = bb // BH
                        nc.gpsimd.collective_compute(
                            "AllToAll", mybir.AluOpType.bypass,
                            replica_groups=[list(range(n_cores))],
                            ins=[a2a_in[i][:].opt()],
                            outs=[a2a_out[i][:].opt()])
                    if box["g"] is not None:
                        drain(box["g"])
                drain(proj_units(1))
    nc.finalize()
    return nc


def make_in_maps(states, Wq, Wk, Wv, Wo, n_cores=N_CORES):
    b_, s_, d_ = states.shape
    R = b_ * s_
    HPC = d_ // n_cores
    bf = ml_dtypes.bfloat16
    statesT = np.ascontiguousarray(
        np.asarray(states, dtype=np.float32).reshape(R, d_).T).astype(bf)
    Wq = np.asarray(Wq, dtype=np.float32).astype(bf)
    Wk = np.asarray(Wk, dtype=np.float32).astype(bf)
    Wv = np.asarray(Wv, dtype=np.float32).astype(bf)
    Wo = np.ascontiguousarray(np.asarray(Wo, dtype=np.float32)).astype(bf)
    tri = build_tri()
    in_maps = []
    for c in range(n_cores):
        in_maps.append({
            "statesT": statesT,
            "wq": np.ascontiguousarray(Wq[:, c * HPC:(c + 1) * HPC]),
            "wk": np.ascontiguousarray(Wk[:, c * HPC:(c + 1) * HPC]),
            "wv": np.ascontiguousarray(Wv[:, c * HPC:(c + 1) * HPC]),
            "wo": Wo,
            "tri": tri,
        })
    return in_maps


def unshard(outs, b_, s_, d_, n_cores=N_CORES):
    """Core j's output rows are [half0: rows HR*j ...][half1: ...]."""
    R = b_ * s_
    Rc = R // n_cores
    n_half = 2
    HR = Rc // n_half
    RH = R // n_half
    full = np.empty((R, d_), dtype=np.float32)
    for j in range(n_cores):
        for hf in range(n_half):
            full[hf * RH + j * HR: hf * RH + (j + 1) * HR] = \
                outs[j][hf * HR:(hf + 1) * HR]
    return full.reshape(b_, s_, d_)


_NC_CACHE = {}


def kernel(states, mask, Wq, Wk, Wv, Wo):
    """Full inputs -> full output [B, S, D]. mask is causal by construction
    (reference builds tril); causality is hardcoded on-chip."""
    from concourse.bass_utils import run_bass_kernel_spmd

    states = np.asarray(states, dtype=np.float32)
    b_, s_, d_ = states.shape
    key = (b_, s_, d_)
    if key not in _NC_CACHE:
        _NC_CACHE[key] = build(b_, s_, d_)
    nc = _NC_CACHE[key]
    in_maps = make_in_maps(states, Wq, Wk, Wv, Wo)
    res = run_bass_kernel_spmd(nc, in_maps, core_ids=list(range(N_CORES)))
    outs = [res.results[c]["out"] for c in range(N_CORES)]
    return unshard(outs, b_, s_, d_).astype(np.float32)
